# revision 4
# baseline (speedup 1.0000x reference)
"""EntityEncoder Trainium2 kernel.

Computes, for each (batch, sentence j): ragged per-entity span mean-pool over
token embeddings, then a Linear projection:

    pooled[b, j, k, :] = mean(zipped_entity[b, j, start_kj:end_kj, :])
    out[b, j*K+k, :]   = pooled @ W + b

Strategy (8 NeuronCores, memory-bound):
  - The unit of work is a sentence (b, j): only tokens [0, R) with
    R = max(sep[b,j,:]) are ever read. All 128 sentences are sorted by R
    and dealt rank-sliced: slot s (0..15) holds sentences ranked
    [8s, 8s+8) -- one per core -- so the shared NEFF's per-slot row
    capacity (the max of 8 nearly-equal R's) wastes ~2% vs the ideal
    rows/8 split. The NEFF is compile-time specialized on the 16
    capacities (cached per tuple).
  - Per core, slots are grouped 4-at-a-time; each group's tokens are
    host-packed into one contiguous [128, C, 768] f16 chunk-transposed
    buffer (f16 halves HBM traffic; ~3.5e-4 rel err). DMA reads exact
    rows (partial final chunk), split into pieces alternating the two
    HWDGE rings; per-partition segments are contiguous runs.
  - Span masks (built host-side) turn the ragged mean-pool into
    mask^T @ Z matmuls on the tensor engine, accumulated over the
    group's chunks into a PSUM pair [16, 384]x2; a chunk may mix tokens
    of several sentences (the 16 mask columns sort it out).
  - sums goes PSUM->SBUF with the two D-halves copied in parallel
    (DVE + ScalarE), is PE-transposed to [768, 16] chunks, scaled by
    1/count in broadcast multiplies (1/0 = inf reproduces the
    reference's 0/0 -> NaN exactly), and contracted with W on
    partitions: out = pooledT.T @ W + b.
"""

import os
import numpy as np
from contextlib import ExitStack

BS, J, L, D = 32, 4, 512, 768
K = 4
OUT = 256
NCORES = 8
NSENT = BS * J          # 128 sentence tasks
NSLOT = NSENT // NCORES  # 16 slots per core
NG = 4                  # sentence groups per core
GS = NSLOT // NG        # slots (sentences) per group = 4
NE = GS * K             # entities per group = 16
DC = D // 128           # 128-row D-chunks = 6
DH = D // 2             # free-dim half for pooling matmul (384 <= 512 psum f32)

# matmul operand dtype: "f16" (default), "f32r", "f32", "bf16"
MM_MODE = os.environ.get("BASSK_MM_MODE", "f16")
# max chunks per z DMA piece (alternating the two HWDGE rings)
ZPIECE = int(os.environ.get("BASSK_ZPIECE", "4"))

_CACHE = {}


def _mm_dt(mybir):
    return {
        "f32r": mybir.dt.float32r,
        "f32": mybir.dt.float32,
        "bf16": mybir.dt.bfloat16,
        "f16": mybir.dt.float16,
    }[MM_MODE]


def _mm_np():
    if MM_MODE == "bf16":
        import ml_dtypes

        return np.dtype(ml_dtypes.bfloat16)
    if MM_MODE == "f16":
        return np.dtype(np.float16)
    return np.dtype(np.float32)


def _geom(caps):
    """Per-group geometry from the 16 slot capacities: token offsets,
    chunk counts, group processing order (ascending size)."""
    caps = tuple(int(c) for c in caps)
    Ts = [sum(caps[g * GS : (g + 1) * GS]) for g in range(NG)]
    Cs = [(t + 127) // 128 for t in Ts]
    cum = np.cumsum([0] + Cs)
    co = [int(c) for c in cum[:-1]]        # chunk offset per group
    ctot = int(cum[-1])
    gorder = sorted(range(NG), key=lambda g: Ts[g])
    return Ts, Cs, co, ctot, gorder


def _build_nc(niter=1, hw_loop=0, nch_sj=None):
    """nch_sj: the 16-tuple of slot row capacities (compile-time plan)."""
    import contextlib

    import concourse.bass as bass
    import concourse.mybir as mybir
    from concourse.bacc import Bacc
    from concourse.tile import TileContext
    from concourse.masks import make_identity

    f32 = mybir.dt.float32
    mmdt = _mm_dt(mybir)
    caps = nch_sj if nch_sj is not None else tuple(L for _ in range(NSLOT))
    Ts, Cs, co, ctot, gorder = _geom(caps)

    nc = Bacc(trn_type="TRN2")
    zp = nc.declare_dram_parameter("zp", [128, ctot, D], mmdt, isOutput=False)
    masks = nc.declare_dram_parameter("masks", [128, ctot * NE], mmdt, isOutput=False)
    rcount = nc.declare_dram_parameter("rcount", [128, NG * NE], f32, isOutput=False)
    brep = nc.declare_dram_parameter("brep", [NE, OUT], f32, isOutput=False)
    w = nc.declare_dram_parameter("w", [D, OUT], mmdt, isOutput=False)
    out = nc.declare_dram_parameter("out", [NG, NE, OUT], f32, isOutput=True)

    with TileContext(nc) as tc:
        with ExitStack() as ctx:
            singles = ctx.enter_context(tc.tile_pool(name="singles", bufs=1))
            zpool = ctx.enter_context(tc.tile_pool(name="zp", bufs=1))
            sums_pool = ctx.enter_context(tc.tile_pool(name="sums", bufs=2))
            ptpool = ctx.enter_context(tc.tile_pool(name="pt", bufs=2))
            outpool = ctx.enter_context(tc.tile_pool(name="outp", bufs=2))
            psum_pool = ctx.enter_context(tc.tile_pool(name="ps", bufs=4, space="PSUM"))
            psum_tp = ctx.enter_context(tc.tile_pool(name="pst", bufs=3, space="PSUM"))
            psum_op = ctx.enter_context(tc.tile_pool(name="pso", bufs=1, space="PSUM"))

            mask_t = singles.tile([128, ctot * NE], mmdt)
            nc.gpsimd.dma_start(out=mask_t, in_=masks[:, :])
            rc_t = singles.tile([128, NG * NE], f32)
            nc.gpsimd.dma_start(out=rc_t, in_=rcount[:, :])
            b_t = singles.tile([NE, OUT], f32)
            nc.gpsimd.dma_start(out=b_t, in_=brep[:, :])
            w_t = singles.tile([128, DC, OUT], mmdt)
            nc.gpsimd.dma_start(out=w_t, in_=w.rearrange("(c p) o -> p c o", p=128))
            ident = singles.tile([NE, NE], f32)
            make_identity(nc, ident)

            rings = (nc.sync, nc.scalar)
            loop_cm = tc.For_i(0, hw_loop, 1) if hw_loop else contextlib.nullcontext()
            with loop_cm:
              for it in range(niter):
                # One resident z tile; all pieces issued up front so the two
                # HWDGE rings stream back-to-back (no compute in their FIFOs).
                # Small first piece (PE starts sooner) and small final piece
                # (shorter end-of-kernel chain).
                zt = zpool.tile([128, ctot, D], mmdt, name=f"zt{it}", tag="zt")
                ring_i = 0
                for gi, g in enumerate(gorder):
                    T, C, CO = Ts[g], Cs[g], co[g]
                    fc, r = divmod(T, 128)
                    bounds = list(range(0, fc, ZPIECE)) + [fc]
                    if gi == 0 and fc > 2:
                        bounds = [0, 2] + [x for x in bounds if x > 2]
                    if gi == NG - 1 and r == 0 and fc > 1:
                        if fc - 1 not in bounds:
                            bounds.insert(-1, fc - 1)
                    for a, b_hi in zip(bounds[:-1], bounds[1:]):
                        rings[ring_i % 2].dma_start(
                            out=zt[:, CO + a : CO + b_hi, :],
                            in_=zp[:, CO + a : CO + b_hi, :],
                        )
                        ring_i += 1
                    if r:
                        rings[ring_i % 2].dma_start(
                            out=zt[0:r, CO + fc, :], in_=zp[0:r, CO + fc, :]
                        )
                        ring_i += 1
                for gi, g in enumerate(gorder):
                    T, C, CO = Ts[g], Cs[g], co[g]
                    fc, r = divmod(T, 128)
                    last_g = gi == NG - 1
                    ps = [
                        psum_pool.tile([NE, DH], f32, name=f"ps{it}_{g}_{h}", tag="ps")
                        for h in range(2)
                    ]
                    for c in range(C):
                        rows = 128 if (c < fc) else r
                        moff = (CO + c) * NE
                        for h in range(2):
                            nc.tensor.matmul(
                                ps[h][:, :],
                                lhsT=mask_t[0:rows, moff : moff + NE],
                                rhs=zt[0:rows, CO + c, h * DH : (h + 1) * DH],
                                start=(c == 0),
                                stop=(c == C - 1),
                            )
                    sums = sums_pool.tile([NE, D], f32, name=f"sums{it}_{g}", tag="sums")
                    nc.vector.tensor_copy(sums[:, 0:DH], ps[0][:, :])
                    # ACT's HWDGE FIFO is only safe to use once all z pieces
                    # are drained (last group); earlier groups keep DVE.
                    if last_g:
                        nc.scalar.copy(sums[:, DH : 2 * DH], ps[1][:, :])
                    else:
                        nc.vector.tensor_copy(sums[:, DH : 2 * DH], ps[1][:, :])
                    pt = ptpool.tile([128, DC, NE], mmdt, name=f"pt{it}_{g}", tag="pt")
                    hdc = DC // 2
                    rc_s = rc_t[:, g * NE : (g + 1) * NE]
                    rc_b = bass.AP(
                        tensor=rc_s.tensor,
                        offset=rc_s.offset,
                        ap=[rc_s.ap[0], [0, hdc], rc_s.ap[1]],
                    )
                    for gg in range(2):
                        tps = psum_tp.tile(
                            [128, hdc, NE], f32, name=f"tps{it}_{g}_{gg}", tag="tp"
                        )
                        for i in range(hdc):
                            dc = gg * hdc + i
                            nc.tensor.transpose(
                                tps[:, i, :], sums[:, dc * 128 : (dc + 1) * 128], ident[:, :]
                            )
                        nc.vector.tensor_mul(
                            pt[:, gg * hdc : (gg + 1) * hdc, :], tps[:, :, :], rc_b
                        )
                    po = psum_op.tile([NE, OUT], f32, name=f"po{it}_{g}", tag="po")
                    for dc in range(DC):
                        nc.tensor.matmul(
                            po[:, :],
                            lhsT=pt[:, dc, :],
                            rhs=w_t[:, dc, :],
                            start=(dc == 0),
                            stop=(dc == DC - 1),
                        )
                    ot = outpool.tile([NE, OUT], f32, name=f"ot{it}_{g}", tag="ot")
                    nc.vector.tensor_add(ot[:, :], po[:, :], b_t[:, :])
                    # SP's HWDGE FIFO must stay clear for z pieces; only the
                    # last group's out (all z drained) uses it.
                    oeng = nc.sync if last_g else nc.gpsimd
                    oeng.dma_start(out=out[g, :, :], in_=ot[:, :])
    nc.finalize()
    return nc


def _plan(sep):
    """Sort sentences by needed rows; slot s gets global ranks [8s, 8s+8).

    Returns (order, caps): order[8s + c] = sentence id for core c slot s;
    caps[s] = slot row capacity (max over its 8 sentences).
    """
    R = np.clip(np.asarray(sep).max(axis=-1).reshape(-1), 1, L).astype(int)  # [128]
    order = np.argsort(-R, kind="stable")
    caps = tuple(int(R[order[s * NCORES]]) for s in range(NSLOT))
    return order, caps


def _prep_in_maps(z, sep, Wf, bf, assign=None):
    # Reference span arithmetic (identical formulas, so edge cases match:
    # count==0 -> 1/0=inf, 0*inf=NaN like the reference's 0/0).
    order = assign if assign is not None else _plan(sep)[0]
    _, caps = _plan(sep)
    Ts, Cs, co, ctot, _ = _geom(caps)

    sep2 = sep.reshape(NSENT, K)
    starts = np.concatenate([np.ones_like(sep2[:, :1]), sep2[:, :-1] + 1], axis=-1)
    ends = sep2
    counts = (ends - starts).astype(np.float32)                  # [128, K]
    with np.errstate(divide="ignore"):
        rcounts = np.float32(1.0) / counts

    mdt = _mm_np()
    brep = np.ascontiguousarray(np.broadcast_to(bf, (NE, OUT)))
    Wm = Wf.astype(mdt, copy=False)
    zflat = z.reshape(NSENT, L, D)

    in_maps = []
    for c in range(NCORES):
        zp = np.zeros((128, ctot, D), mdt)
        mfull = np.zeros((128, ctot, NE), mdt)
        rc = np.zeros((NG, NE), np.float32)
        for g in range(NG):
            T, C, CO = Ts[g], Cs[g], co[g]
            ztok = np.zeros((C * 128, D), mdt)
            mtok = np.zeros((C * 128, NE), mdt)
            off = 0
            for jslot in range(GS):
                s = g * GS + jslot
                sid = int(order[s * NCORES + c])
                cap = caps[s]
                Rown = min(int(np.clip(sep2[sid].max(), 1, L)), cap)
                ztok[off : off + Rown] = zflat[sid, :Rown].astype(mdt)
                l = np.arange(cap)
                m = (l[:, None] >= starts[sid][None, :]) & (
                    l[:, None] < ends[sid][None, :]
                )
                mtok[off : off + cap, jslot * K : (jslot + 1) * K] = m.astype(mdt)
                rc[g, jslot * K : (jslot + 1) * K] = rcounts[sid]
                off += cap
            zp[:, CO : CO + C, :] = ztok.reshape(C, 128, D).transpose(1, 0, 2)
            mfull[:, CO : CO + C, :] = mtok.reshape(C, 128, NE).transpose(1, 0, 2)
        in_maps.append(
            {
                "zp": zp,
                "masks": np.ascontiguousarray(mfull.reshape(128, ctot * NE)),
                "rcount": np.ascontiguousarray(
                    np.broadcast_to(rc.reshape(1, NG * NE), (128, NG * NE))
                ),
                "brep": brep,
                "w": Wm,
            }
        )
    return in_maps


def _run(in_maps, nch_sj=None, **kwargs):
    from concourse.bass_utils import run_bass_kernel_spmd

    key = ("nc", nch_sj, MM_MODE)
    if key not in _CACHE:
        _CACHE[key] = _build_nc(nch_sj=nch_sj)
    return run_bass_kernel_spmd(_CACHE[key], in_maps, list(range(NCORES)), **kwargs)


def kernel(zipped_entity, entity_token_sep_idx, W, b):
    z = np.ascontiguousarray(np.asarray(zipped_entity, dtype=np.float32))
    sep = np.asarray(entity_token_sep_idx).astype(np.int64)
    Wf = np.ascontiguousarray(np.asarray(W, dtype=np.float32))
    bf = np.asarray(b, dtype=np.float32)
    assert z.shape == (BS, J, L, D) and sep.shape == (BS, J, K)

    order, caps = _plan(sep)
    res = _run(_prep_in_maps(z, sep, Wf, bf, assign=order), nch_sj=caps)
    out = np.empty((BS, J * K, OUT), np.float32)
    for c in range(NCORES):
        oc = res.results[c]["out"]                # [NG, NE, OUT]
        for s in range(NSLOT):
            g, jslot = divmod(s, GS)
            sid = int(order[s * NCORES + c])
            bb, jj = divmod(sid, J)
            out[bb, jj * K : (jj + 1) * K] = oc[g, jslot * K : (jslot + 1) * K]
    return out
